# revision 44
# baseline (speedup 1.0000x reference)
"""Trainium2 Bass kernel for nn_CausalSelfAttention (GQA + RoPE + qk-RMSNorm).

Strategy (Megatron-style head parallelism over 8 NeuronCores):
  - Each core owns 2 of the 16 q heads and the matching 1 of 8 kv heads.
  - Per core: QKV projection for its 512 rows of w_attn, RoPE + qk RMS norm,
    causal flash-style attention for its (2 q heads x 2 batches), and a
    partial output projection through its 256 columns of w_proj.
  - Host sums the 8 partial outputs (no on-device collectives).

All tensors are fed to the device pre-swizzled into SBUF-ready
[128, free...] layouts (bf16 for matmul operands).  Matmuls run in bf16 with
fp32 PSUM accumulation; softmax/statistics run in fp32.

Self-contained: hardcodes all shapes from the problem spec.
"""

import math
import numpy as np
import ml_dtypes
from contextlib import ExitStack

# ---- problem constants (hardcoded per spec) ----
B, T, C = 2, 2048, 2048
N_HEAD, N_KV_HEAD, HD = 16, 8, 128
KV_DIM = N_KV_HEAD * HD
EPS = 1.1920929e-07
N_CORES = 8
QH_PER_CORE = N_HEAD // N_CORES          # 2
TOK = B * T                              # 4096
P = 128
TG = 512                                 # token group (matmul N)
NT = TOK // TG                           # 8 token groups
KT = C // P                              # 16 contraction tiles
NGB = T // TG                            # 4 q groups per batch
NJB = T // P                             # 16 k tiles per batch
SCALE = 1.0 / math.sqrt(HD)

BF16 = ml_dtypes.bfloat16

_CACHE = {}


# --------------------------------------------------------------------------
# device program
# --------------------------------------------------------------------------

def _emit(tc, out_ap, t_in):
    import concourse.bass as bass  # noqa: F401
    import concourse.mybir as mybir

    f32 = mybir.dt.float32
    bf16 = mybir.dt.bfloat16
    AF = mybir.ActivationFunctionType
    nc = tc.nc

    x_d = t_in["x_sw"]
    wq_d = t_in["wq_sw"]
    wp_d = t_in["wp_sw"]
    cs_d = t_in["cs_sw"]
    mask_d = t_in["mask_sw"]
    eye_d = t_in["eye_sw"]
    ones_d = t_in["ones_sw"]

    with ExitStack() as root:
        const = root.enter_context(tc.tile_pool(name="const", bufs=1))
        # startup: interleave x(n=0) k-chunks with per-m weight chunks so the
        # first QKV chain starts ~2.5us in and is never DMA-gated afterwards.
        # wq layout is [P, m, k, 128]: per-m slices are contiguous.
        wq_sb = const.tile([P, 4, KT, P], bf16)
        x0_sb = const.tile([P, KT, TG], bf16, tag="x0")
        # finest-grain interleave for the first two chains (m=2 and m=0 run
        # in half-k alternation, so PE has work while x0's tail streams in)
        nc.sync.dma_start(out=x0_sb[:, 0:4, :], in_=x_d[:, 0, 0:4, :])
        nc.sync.dma_start(out=wq_sb[:, 2, 0:8], in_=wq_d[:, 2, 0:8])
        nc.sync.dma_start(out=wq_sb[:, 0, 0:8], in_=wq_d[:, 0, 0:8])
        nc.sync.dma_start(out=x0_sb[:, 4:8, :], in_=x_d[:, 0, 4:8, :])
        nc.sync.dma_start(out=wq_sb[:, 2, 8:16], in_=wq_d[:, 2, 8:16])
        nc.sync.dma_start(out=wq_sb[:, 0, 8:16], in_=wq_d[:, 0, 8:16])
        nc.sync.dma_start(out=x0_sb[:, 8:16, :], in_=x_d[:, 0, 8:16, :])
        nc.sync.dma_start(out=wq_sb[:, 1], in_=wq_d[:, 1])
        nc.sync.dma_start(out=wq_sb[:, 3], in_=wq_d[:, 3])
        ones_sb = const.tile([P, 1], bf16)
        nc.sync.dma_start(out=ones_sb[:], in_=ones_d)
        eye_sb = const.tile([P, P], bf16)
        nc.sync.dma_start(out=eye_sb[:], in_=eye_d)
        cs_sb = const.tile([P, 2, T], bf16)
        nc.sync.dma_start(out=cs_sb[:], in_=cs_d)
        mask_sb = const.tile([P, 4, TG], bf16)
        nc.sync.dma_start(out=mask_sb[:], in_=mask_d)
        wp_sb = const.tile([P, QH_PER_CORE, C], bf16)
        nc.sync.dma_start(out=wp_sb[:], in_=wp_d)
        eps_sb = const.tile([P, 1], f32)
        nc.vector.memset(eps_sb[:], EPS)
        onesm_sb = const.tile([P, P], bf16)
        nc.vector.memset(onesm_sb[:], 1.0)

        big = root.enter_context(tc.tile_pool(name="big", bufs=1))
        # post-rope, post-norm q (2 heads) and k, in [d, tok] layout
        qn = [big.tile([P, TOK], bf16, name=f"qn{m}", tag=f"qn{m}") for m in range(3)]
        v_sb = big.tile([P, TOK], bf16, tag="v")
        vT_sb = big.tile([P, 2 * NJB, P], bf16, tag="vT")   # [ktok, (b,j), d]
        yT = [big.tile([P, TOK], bf16, name=f"yT{h}", tag=f"yT{h}") for h in range(QH_PER_CORE)]

        # ------- stage 1+2: QKV projection + rope/norm + v transpose -------
        # rope runs at (n, m) granularity right after each normalized tile so
        # nothing serializes at the batch boundary before attention starts.
        with ExitStack() as s1:
            xin = s1.enter_context(tc.tile_pool(name="xin", bufs=2))
            qkv_ps = s1.enter_context(tc.tile_pool(name="qkvps", bufs=3, space="PSUM"))
            vt_ps = s1.enter_context(tc.tile_pool(name="vtps", bufs=3, space="PSUM"))
            ssq_ps = s1.enter_context(tc.tile_pool(name="ssqps", bufs=2, space="PSUM"))
            sqp = s1.enter_context(tc.tile_pool(name="sq", bufs=3))
            srp = s1.enter_context(tc.tile_pool(name="sr", bufs=3))
            ropet = s1.enter_context(tc.tile_pool(name="ropet", bufs=4))

            for b in range(B):
                for nn in range(NT // B):
                    n = b * (NT // B) + nn
                    tsl = slice(n * TG, (n + 1) * TG)       # token cols in qn
                    csl = slice(nn * TG, (nn + 1) * TG)     # cos/sin cols
                    if n == 0:
                        xb = x0_sb
                    else:
                        xb = xin.tile([P, KT, TG], bf16)
                        nc.sync.dma_start(out=xb[:, 0:8, :], in_=x_d[:, n, 0:8, :])
                        nc.sync.dma_start(out=xb[:, 8:16, :], in_=x_d[:, n, 8:16, :])
                    rope_pend = []

                    def _rope_flush(lo=2):
                        # finish rope (xsw*sin + add) one m-step late, so the
                        # vector queue never head-blocks on the half-swap DMA
                        while len(rope_pend) >= lo:
                            mm_, t1_, xsw_, tsl_ = rope_pend.pop(0)
                            nc.vector.tensor_mul(
                                xsw_[:], xsw_[:], cs_sb[:, 1, csl])
                            nc.vector.tensor_add(
                                qn[mm_][:, tsl_], t1_[:], xsw_[:])

                    def _chain(m, ps, ks):
                        for k in ks:
                            nc.tensor.matmul(
                                ps[:],
                                wq_sb[:, m, k],
                                xb[:, k],
                                start=(k == 0),
                                stop=(k == KT - 1),
                            )

                    n0_ps = {}
                    if n == 0:
                        # half-k interleave of the first two chains: PE stays
                        # fed while the second half of x0 is still in flight
                        for m in (2, 0):
                            n0_ps[m] = qkv_ps.tile([P, TG], f32, name="ps")
                            _chain(m, n0_ps[m], range(0, 8))
                        for m in (2, 0):
                            _chain(m, n0_ps[m], range(8, KT))

                    for m in (2, 0, 1, 3):
                        if m in n0_ps:
                            ps = n0_ps[m]
                        else:
                            ps = qkv_ps.tile([P, TG], f32, name="ps")
                            _chain(m, ps, range(KT))
                        if m == 3:
                            # off the (busy) vector queue; ACT reads PSUM fast
                            nc.scalar.copy(v_sb[:, tsl], ps[:])
                        else:
                            # rms-norm: broadcast sum-of-squares via all-ones MM
                            sq = sqp.tile([P, TG], bf16)
                            nc.scalar.activation(sq[:], ps[:], AF.Square)
                            ssqb = ssq_ps.tile([P, TG], f32)
                            nc.tensor.matmul(
                                ssqb[:], onesm_sb[:], sq[:], start=True, stop=True
                            )
                            srb = srp.tile([P, TG], f32)
                            nc.scalar.activation(
                                srb[:], ssqb[:], AF.Sqrt,
                                bias=eps_sb[:], scale=1.0 / HD,
                            )
                            nc.vector.reciprocal_approx_fast(srb[:], srb[:])
                            # normalized copy psum -> sbuf (rope rotation
                            # commutes with the per-token scale)
                            nc.vector.tensor_mul(qn[m][:, tsl], ps[:], srb[:])
                            # rope on this 512-token tile; the half-swap runs
                            # on the sync HWDGE queue, t1 = [x1*c ; x2*c] now,
                            # xsw*s2n = [x2*s ; -x1*s] deferred one m-step
                            t1 = ropet.tile([P, TG], bf16, tag="t1")
                            xsw = ropet.tile([P, TG], bf16, tag="xsw")
                            nc.gpsimd.dma_start(
                                out=xsw[0:64, :], in_=qn[m][64:128, tsl])
                            nc.gpsimd.dma_start(
                                out=xsw[64:128, :], in_=qn[m][0:64, tsl])
                            nc.vector.tensor_mul(
                                t1[:], qn[m][:, tsl], cs_sb[:, 0, csl])
                            rope_pend.append((m, t1, xsw, tsl))
                            _rope_flush(2)
                        if m == 3:
                            _rope_flush(1)
                    # v transposes deferred by one n group so PE never waits
                    # on the v copy trailing in the scalar queue
                    def _vt(nv):
                        for blk in range(nv * (TG // P), (nv + 1) * (TG // P)):
                            tp = vt_ps.tile([P, P], bf16)
                            nc.tensor.transpose(
                                tp[:], v_sb[:, blk * P:(blk + 1) * P], eye_sb[:]
                            )
                            if blk % 2:
                                nc.vector.tensor_copy(vT_sb[:, blk], tp[:])
                            else:
                                nc.scalar.copy(vT_sb[:, blk], tp[:])
                    if n > 0:
                        _vt(n - 1)
                    if n == NT - 1:
                        _vt(n)

        # ---------------- stage 3+4: attention + output projection --------
        # Units of (batch, 512-token q group), software-pipelined ACROSS
        # units:  [pairs(X)] [proj(X-1)] [tail(X)] [pairs(X+1)] [proj(X)] ...
        # pairs(X) = scores (PE) -> exp (ACT) -> attn@V (PE) at pair
        # granularity for both heads; proj(X-1) gives PE ~7us of
        # exp-independent matmuls so ACT drains stragglers; tail(X) =
        # denominator ones-matmuls (fed by vector pair/quad pre-sums of the
        # exp tiles) + reciprocal + normalize.  Diagonal tiles are
        # column-trimmed everywhere.
        with ExitStack() as s3:
            s_ps = s3.enter_context(tc.tile_pool(name="sps", bufs=2, space="PSUM"))
            y_ps = s3.enter_context(tc.tile_pool(name="yps", bufs=1, space="PSUM"))
            o_ps = s3.enter_context(tc.tile_pool(name="ops", bufs=2, space="PSUM"))
            ptp = s3.enter_context(tc.tile_pool(name="pt", bufs=16))
            psm = s3.enter_context(tc.tile_pool(name="psm", bufs=12))
            denp = s3.enter_context(tc.tile_pool(name="den", bufs=2))
            ostgp = s3.enter_context(tc.tile_pool(name="ostg", bufs=3))

            def emit_pairs(b, g, qh, yp):
                """scores/exp/attn@V pair pipeline; returns tail state."""
                q_t, k_t = qn[qh], qn[2]
                qsl = slice(b * T + g * TG, b * T + (g + 1) * TG)
                jmax = 4 * g + 3

                def issue_y(j, ap, off):
                    nc.tensor.matmul(
                        yp[:, off:] if off else yp[:],
                        vT_sb[:, b * NJB + j],
                        ap[:, off:] if off else ap[:],
                        start=(j == 0), stop=(j == jmax),
                    )

                pend, pairs, quads, diags = [], [], [], []
                for pr in range((jmax + 1) // 2):
                    sp2 = s_ps.tile([P, 2, TG], f32)
                    for jj in (0, 1):
                        j = 2 * pr + jj
                        off = (j - 4 * g) * P if j >= 4 * g else 0
                        nc.tensor.matmul(
                            sp2[:, jj, off:],
                            k_t[:, b * T + j * P: b * T + (j + 1) * P],
                            q_t[:, qsl][:, off:],
                            start=True, stop=True,
                        )
                    pt2 = ptp.tile([P, 2, TG], bf16)
                    cur = []
                    if 2 * pr >= 4 * g:
                        # diagonal pair: per-j exp on the valid range only
                        for jj in (0, 1):
                            j = 2 * pr + jj
                            off = (j - 4 * g) * P
                            nc.scalar.activation(
                                pt2[:, jj, off:], sp2[:, jj, off:],
                                AF.Exp, scale=SCALE,
                            )
                            nc.gpsimd.tensor_mul(
                                pt2[:, jj, off:off + P],
                                pt2[:, jj, off:off + P],
                                mask_sb[:, 0, 0:P],
                            )
                            cur.append((j, pt2[:, jj], off))
                            diags.append((j, pt2[:, jj], off))
                    else:
                        nc.scalar.activation(pt2[:], sp2[:], AF.Exp,
                                             scale=SCALE)
                        cur.append((2 * pr, pt2[:, 0], 0))
                        cur.append((2 * pr + 1, pt2[:, 1], 0))
                        # den partials: vector pre-sums (pair, then quad)
                        psum = psm.tile([P, TG], bf16, tag="pair")
                        nc.vector.tensor_add(psum[:], pt2[:, 0], pt2[:, 1])
                        pairs.append(psum)
                        if len(pairs) == 2:
                            qd = psm.tile([P, TG], bf16, tag="quad")
                            nc.vector.tensor_add(
                                qd[:], pairs[0][:], pairs[1][:])
                            quads.append(qd)
                            pairs = []
                    # attn@V for the previous pair while ACT runs this exp
                    if pend:
                        for (j, ap, off) in pend.pop(0):
                            issue_y(j, ap, off)
                    pend.append(cur)
                return (b, g, qh, yp, pend, quads, diags, jmax, qsl, issue_y)

            def emit_tail(st):
                """trailing attn@V + denominator + normalize for one head."""
                b, g, qh, yp, pend, quads, diags, jmax, qsl, issue_y = st
                for grp in pend:
                    for (j, ap, off) in grp:
                        issue_y(j, ap, off)
                dp = o_ps.tile([P, TG], f32, tag="op")
                first = True
                for qd in quads:
                    nc.tensor.matmul(dp[:], onesm_sb[:], qd[:],
                                     start=first, stop=False)
                    first = False
                for (j, ap, off) in diags:
                    nc.tensor.matmul(
                        dp[:, off:] if off else dp[:],
                        onesm_sb[:],
                        ap[:, off:] if off else ap[:],
                        start=first, stop=(j == jmax),
                    )
                    first = False
                den = denp.tile([P, TG], f32)
                nc.vector.reciprocal_approx_fast(den[:], dp[:])
                nc.vector.tensor_mul(yT[qh][:, qsl], yp[:], den[:])

            def emit_proj(b, g, last=False):
                dense = (b == B - 1 and g <= 1) or last
                for tt in range(b * (T // P) + g * 4,
                                b * (T // P) + g * 4 + 4):
                    ost = ostgp.tile([P, C], bf16)
                    for og in range(C // TG):
                        op = o_ps.tile([P, TG], f32, tag="op")
                        nc.tensor.matmul(
                            op[:], yT[0][:, tt * P:(tt + 1) * P],
                            wp_sb[:, 0, og * TG:(og + 1) * TG],
                            start=True, stop=False,
                        )
                        nc.tensor.matmul(
                            op[:], yT[1][:, tt * P:(tt + 1) * P],
                            wp_sb[:, 1, og * TG:(og + 1) * TG],
                            start=False, stop=True,
                        )
                        # copies mostly on vector (scalar stays clear for
                        # exp); final unit splits evenly to shorten the tail
                        if og % 2 if dense else og == 1:
                            nc.scalar.copy(
                                ost[:, og * TG:(og + 1) * TG], op[:])
                        else:
                            nc.vector.tensor_copy(
                                ost[:, og * TG:(og + 1) * TG], op[:])
                        if last:
                            # per-og DMA so the final writes overlap copies
                            nc.sync.dma_start(
                                out=out_ap[tt * P:(tt + 1) * P,
                                           og * TG:(og + 1) * TG],
                                in_=ost[:, og * TG:(og + 1) * TG])
                    if not last:
                        nc.sync.dma_start(
                            out=out_ap[tt * P:(tt + 1) * P, :], in_=ost[:])

            units = [(b, g) for b in range(B) for g in (3, 2, 1, 0)]
            prev = None
            for (b, g) in units:
                yp0 = y_ps.tile([P, TG], f32, tag="yp0")
                yp1 = y_ps.tile([P, TG], f32, tag="yp1")
                st0 = emit_pairs(b, g, 0, yp0)
                st1 = emit_pairs(b, g, 1, yp1)
                # head-0 tail first so its recip/normalize enter the vector
                # queue ahead of the previous unit's 12 projection copies
                emit_tail(st0)
                if prev is not None:
                    emit_proj(*prev)
                emit_tail(st1)
                prev = (b, g)
            emit_proj(*prev, last=True)

def build_nc():
    """Build and compile the (single, shared across cores) Bass program."""
    if "nc" in _CACHE:
        return _CACHE["nc"]
    import concourse.mybir as mybir
    import concourse.tile as tile
    from concourse import bacc

    f32 = mybir.dt.float32  # noqa: F841
    bf16 = mybir.dt.bfloat16

    nc = bacc.Bacc("TRN2", target_bir_lowering=False, debug=False)
    shapes = {
        "x_sw": ((P, NT, KT, TG), bf16),
        "wq_sw": ((P, 4, KT, P), bf16),
        "wp_sw": ((P, QH_PER_CORE, C), bf16),
        "cs_sw": ((P, 2, T), bf16),
        "mask_sw": ((P, 4, TG), bf16),
        "eye_sw": ((P, P), bf16),
        "ones_sw": ((P, 1), bf16),
    }
    t_in = {
        name: nc.dram_tensor(name, shape, dt, kind="ExternalInput").ap()
        for name, (shape, dt) in shapes.items()
    }
    out_ap = nc.dram_tensor("out", (TOK, C), bf16, kind="ExternalOutput").ap()

    with tile.TileContext(nc) as tc:
        _emit(tc, out_ap, t_in)
    nc.compile()
    _CACHE["nc"] = nc
    return nc


# --------------------------------------------------------------------------
# host-side data preparation
# --------------------------------------------------------------------------

def _swizzle_ktiles(a2d):
    """[R*128, F] -> [128, R, F] picking partition-within-tile as leading."""
    r128, f = a2d.shape
    r = r128 // P
    return np.ascontiguousarray(a2d.reshape(r, P, f).transpose(1, 0, 2))


def host_prep(x, w_attn, w_proj, cos, sin):
    x = np.asarray(x, np.float32)
    w_attn = np.asarray(w_attn, np.float32)
    w_proj = np.asarray(w_proj, np.float32)
    cos = np.asarray(cos, np.float32).reshape(T, HD // 2)
    sin = np.asarray(sin, np.float32).reshape(T, HD // 2)

    # x: (B,T,C) -> xT (C, TOK) -> [128, n, k, t]
    xT = x.reshape(TOK, C).T                        # (C, TOK)
    x_sw = (
        xT.reshape(KT, P, NT, TG).transpose(1, 2, 0, 3)  # (P, n, k, t)
    )
    x_sw = np.ascontiguousarray(x_sw).astype(BF16)

    # cos/sin duplicated across both 64-partition halves: [128, 2, T]
    c2 = np.concatenate([cos.T, cos.T], axis=0)     # (128, T)
    s2 = np.concatenate([sin.T, -sin.T], axis=0)    # sign-folded for rope add
    cs_sw = np.stack([c2, s2], axis=1).astype(BF16)  # (128, 2, T)

    # causal masks for the 4 diagonal offsets: keep col >= row + 128*off
    col = np.arange(TG)[None, :]
    row = np.arange(P)[:, None]
    mask_sw = np.stack(
        [(col >= row + P * off) for off in range(4)], axis=1
    ).astype(BF16)                                   # (128, 4, 512)

    eye_sw = np.eye(P, dtype=np.float32).astype(BF16)
    ones_sw = np.ones((P, 1), np.float32).astype(BF16)

    in_maps = []
    for c in range(N_CORES):
        qrows = w_attn[QH_PER_CORE * HD * c: QH_PER_CORE * HD * (c + 1)]
        krows = w_attn[C + HD * c: C + HD * (c + 1)]
        vrows = w_attn[C + KV_DIM + HD * c: C + KV_DIM + HD * (c + 1)]
        w_sel = np.concatenate([qrows, krows, vrows], axis=0)   # (512, C)
        wq_sw = _swizzle_ktiles(w_sel.T).astype(BF16)           # (128, 16, 512)
        # [p, k, m*128+c] -> [p, m, k, c] (per-m contiguous for startup DMA)
        wq_sw = np.ascontiguousarray(
            wq_sw.reshape(P, KT, 4, P).transpose(0, 2, 1, 3))

        wp_sel = w_proj[:, QH_PER_CORE * HD * c: QH_PER_CORE * HD * (c + 1)]
        wp_sw = _swizzle_ktiles(np.ascontiguousarray(wp_sel.T)).astype(BF16)

        in_maps.append({
            "x_sw": x_sw,
            "wq_sw": wq_sw,
            "wp_sw": np.ascontiguousarray(wp_sw.reshape(P, QH_PER_CORE, C)),
            "cs_sw": cs_sw,
            "mask_sw": mask_sw,
            "eye_sw": eye_sw,
            "ones_sw": ones_sw,
        })
    return in_maps


def run_on_hw(in_maps, trace=False, **kwargs):
    from concourse import bass_utils

    nc = build_nc()
    return bass_utils.run_bass_kernel_spmd(
        nc, in_maps, core_ids=list(range(N_CORES)), trace=trace, **kwargs
    )


def kernel(x, w_attn, w_proj, cos, sin):
    in_maps = host_prep(x, w_attn, w_proj, cos, sin)
    res = run_on_hw(in_maps)
    out = np.zeros((TOK, C), np.float64)
    for r in res.results:
        out += r["out"].astype(np.float64)
    return out.astype(np.float32).reshape(B, T, C)



# revision 45
# speedup vs baseline: 1.1696x; 1.1696x over previous
"""Trainium2 Bass kernel for nn_CausalSelfAttention (GQA + RoPE + qk-RMSNorm).

Strategy (Megatron-style head parallelism over 8 NeuronCores):
  - Each core owns 2 of the 16 q heads and the matching 1 of 8 kv heads.
  - Per core: QKV projection for its 512 rows of w_attn, RoPE + qk RMS norm,
    causal flash-style attention for its (2 q heads x 2 batches), and a
    partial output projection through its 256 columns of w_proj.
  - Host sums the 8 partial outputs (no on-device collectives).

All tensors are fed to the device pre-swizzled into SBUF-ready
[128, free...] layouts (bf16 for matmul operands).  Matmuls run in bf16 with
fp32 PSUM accumulation; softmax/statistics run in fp32.

Self-contained: hardcodes all shapes from the problem spec.
"""

import math
import numpy as np
import ml_dtypes
from contextlib import ExitStack

# ---- problem constants (hardcoded per spec) ----
B, T, C = 2, 2048, 2048
N_HEAD, N_KV_HEAD, HD = 16, 8, 128
KV_DIM = N_KV_HEAD * HD
EPS = 1.1920929e-07
N_CORES = 8
QH_PER_CORE = N_HEAD // N_CORES          # 2
TOK = B * T                              # 4096
P = 128
TG = 512                                 # token group (matmul N)
NT = TOK // TG                           # 8 token groups
KT = C // P                              # 16 contraction tiles
NGB = T // TG                            # 4 q groups per batch
NJB = T // P                             # 16 k tiles per batch
SCALE = 1.0 / math.sqrt(HD)

BF16 = ml_dtypes.bfloat16

_CACHE = {}


# --------------------------------------------------------------------------
# device program
# --------------------------------------------------------------------------

def _emit(tc, out_ap, t_in):
    import concourse.bass as bass  # noqa: F401
    import concourse.mybir as mybir

    f32 = mybir.dt.float32
    bf16 = mybir.dt.bfloat16
    AF = mybir.ActivationFunctionType
    nc = tc.nc

    x_d = t_in["x_sw"]
    wq_d = t_in["wq_sw"]
    wp_d = t_in["wp_sw"]
    cs_d = t_in["cs_sw"]
    mask_d = t_in["mask_sw"]
    eye_d = t_in["eye_sw"]
    ones_d = t_in["ones_sw"]

    with ExitStack() as root:
        const = root.enter_context(tc.tile_pool(name="const", bufs=1))
        # startup: interleave x(n=0) k-chunks with per-m weight chunks so the
        # first QKV chain starts ~2.5us in and is never DMA-gated afterwards.
        # wq layout is [P, m, k, 128]: per-m slices are contiguous.
        wq_sb = const.tile([P, 4, KT, P], bf16)
        x0_sb = const.tile([P, KT, TG], bf16, tag="x0")
        # finest-grain interleave for the first two chains (m=2 and m=0 run
        # in half-k alternation, so PE has work while x0's tail streams in)
        nc.sync.dma_start(out=x0_sb[:, 0:4, :], in_=x_d[:, 0, 0:4, :])
        nc.sync.dma_start(out=wq_sb[:, 2, 0:8], in_=wq_d[:, 2, 0:8])
        nc.sync.dma_start(out=wq_sb[:, 0, 0:8], in_=wq_d[:, 0, 0:8])
        nc.sync.dma_start(out=x0_sb[:, 4:8, :], in_=x_d[:, 0, 4:8, :])
        nc.sync.dma_start(out=wq_sb[:, 2, 8:16], in_=wq_d[:, 2, 8:16])
        nc.sync.dma_start(out=wq_sb[:, 0, 8:16], in_=wq_d[:, 0, 8:16])
        nc.sync.dma_start(out=x0_sb[:, 8:16, :], in_=x_d[:, 0, 8:16, :])
        nc.sync.dma_start(out=wq_sb[:, 1], in_=wq_d[:, 1])
        nc.sync.dma_start(out=wq_sb[:, 3], in_=wq_d[:, 3])
        ones_sb = const.tile([P, 1], bf16)
        nc.sync.dma_start(out=ones_sb[:], in_=ones_d)
        eye_sb = const.tile([P, P], bf16)
        nc.sync.dma_start(out=eye_sb[:], in_=eye_d)
        cs_sb = const.tile([P, 2, T], bf16)
        nc.sync.dma_start(out=cs_sb[:], in_=cs_d)
        mask_sb = const.tile([P, 4, TG], bf16)
        nc.sync.dma_start(out=mask_sb[:], in_=mask_d)
        wp_sb = const.tile([P, QH_PER_CORE, C], bf16)
        nc.sync.dma_start(out=wp_sb[:], in_=wp_d)
        eps_sb = const.tile([P, 1], f32)
        nc.vector.memset(eps_sb[:], EPS)
        onesm_sb = const.tile([P, P], bf16)
        nc.vector.memset(onesm_sb[:], 1.0)

        big = root.enter_context(tc.tile_pool(name="big", bufs=1))
        # post-rope, post-norm q (2 heads) and k, in [d, tok] layout
        qn = [big.tile([P, TOK], bf16, name=f"qn{m}", tag=f"qn{m}") for m in range(3)]
        v_sb = big.tile([P, TOK], bf16, tag="v")
        vT_sb = big.tile([P, 2 * NJB, P], bf16, tag="vT")   # [ktok, (b,j), d]
        yT = [big.tile([P, TOK], bf16, name=f"yT{h}", tag=f"yT{h}") for h in range(QH_PER_CORE)]

        # ------- stage 1+2: QKV projection + rope/norm + v transpose -------
        # rope runs at (n, m) granularity right after each normalized tile so
        # nothing serializes at the batch boundary before attention starts.
        with ExitStack() as s1:
            xin = s1.enter_context(tc.tile_pool(name="xin", bufs=2))
            qkv_ps = s1.enter_context(tc.tile_pool(name="qkvps", bufs=3, space="PSUM"))
            vt_ps = s1.enter_context(tc.tile_pool(name="vtps", bufs=3, space="PSUM"))
            ssq_ps = s1.enter_context(tc.tile_pool(name="ssqps", bufs=2, space="PSUM"))
            sqp = s1.enter_context(tc.tile_pool(name="sq", bufs=3))
            srp = s1.enter_context(tc.tile_pool(name="sr", bufs=3))
            ropet = s1.enter_context(tc.tile_pool(name="ropet", bufs=4))

            for b in range(B):
                for nn in range(NT // B):
                    n = b * (NT // B) + nn
                    tsl = slice(n * TG, (n + 1) * TG)       # token cols in qn
                    csl = slice(nn * TG, (nn + 1) * TG)     # cos/sin cols
                    if n == 0:
                        xb = x0_sb
                    else:
                        xb = xin.tile([P, KT, TG], bf16)
                        nc.sync.dma_start(out=xb[:, 0:8, :], in_=x_d[:, n, 0:8, :])
                        nc.sync.dma_start(out=xb[:, 8:16, :], in_=x_d[:, n, 8:16, :])
                    rope_pend = []

                    def _rope_flush(lo=2):
                        # finish rope (xsw*sin + add) one m-step late, so the
                        # vector queue never head-blocks on the half-swap DMA
                        while len(rope_pend) >= lo:
                            mm_, t1_, xsw_, tsl_ = rope_pend.pop(0)
                            nc.vector.tensor_mul(
                                xsw_[:], xsw_[:], cs_sb[:, 1, csl])
                            nc.vector.tensor_add(
                                qn[mm_][:, tsl_], t1_[:], xsw_[:])

                    def _chain(m, ps, ks):
                        for k in ks:
                            nc.tensor.matmul(
                                ps[:],
                                wq_sb[:, m, k],
                                xb[:, k],
                                start=(k == 0),
                                stop=(k == KT - 1),
                            )

                    n0_ps = {}
                    if n == 0:
                        # half-k interleave of the first two chains: PE stays
                        # fed while the second half of x0 is still in flight
                        for m in (2, 0):
                            n0_ps[m] = qkv_ps.tile([P, TG], f32, name="ps")
                            _chain(m, n0_ps[m], range(0, 8))
                        for m in (2, 0):
                            _chain(m, n0_ps[m], range(8, KT))

                    for m in (2, 0, 1, 3):
                        if m in n0_ps:
                            ps = n0_ps[m]
                        else:
                            ps = qkv_ps.tile([P, TG], f32, name="ps")
                            _chain(m, ps, range(KT))
                        if m == 3:
                            # off the (busy) vector queue; ACT reads PSUM fast
                            nc.scalar.copy(v_sb[:, tsl], ps[:])
                        else:
                            # rms-norm: broadcast sum-of-squares via all-ones MM
                            sq = sqp.tile([P, TG], bf16)
                            nc.scalar.activation(sq[:], ps[:], AF.Square)
                            ssqb = ssq_ps.tile([P, TG], f32)
                            nc.tensor.matmul(
                                ssqb[:], onesm_sb[:], sq[:], start=True, stop=True
                            )
                            srb = srp.tile([P, TG], f32)
                            nc.scalar.activation(
                                srb[:], ssqb[:], AF.Sqrt,
                                bias=eps_sb[:], scale=1.0 / HD,
                            )
                            nc.vector.reciprocal_approx_fast(srb[:], srb[:])
                            # normalized copy psum -> sbuf (rope rotation
                            # commutes with the per-token scale)
                            nc.vector.tensor_mul(qn[m][:, tsl], ps[:], srb[:])
                            # rope on this 512-token tile; the half-swap runs
                            # on the sync HWDGE queue, t1 = [x1*c ; x2*c] now,
                            # xsw*s2n = [x2*s ; -x1*s] deferred one m-step
                            t1 = ropet.tile([P, TG], bf16, tag="t1")
                            xsw = ropet.tile([P, TG], bf16, tag="xsw")
                            nc.gpsimd.dma_start(
                                out=xsw[0:64, :], in_=qn[m][64:128, tsl])
                            nc.gpsimd.dma_start(
                                out=xsw[64:128, :], in_=qn[m][0:64, tsl])
                            nc.vector.tensor_mul(
                                t1[:], qn[m][:, tsl], cs_sb[:, 0, csl])
                            rope_pend.append((m, t1, xsw, tsl))
                            _rope_flush(2)
                        if m == 3:
                            _rope_flush(1)
                    # v transposes deferred by one n group so PE never waits
                    # on the v copy trailing in the scalar queue
                    def _vt(nv):
                        for blk in range(nv * (TG // P), (nv + 1) * (TG // P)):
                            tp = vt_ps.tile([P, P], bf16)
                            nc.tensor.transpose(
                                tp[:], v_sb[:, blk * P:(blk + 1) * P], eye_sb[:]
                            )
                            if blk % 2:
                                nc.vector.tensor_copy(vT_sb[:, blk], tp[:])
                            else:
                                nc.scalar.copy(vT_sb[:, blk], tp[:])
                    if n > 0:
                        _vt(n - 1)
                    if n == NT - 1:
                        _vt(n)

        # ---------------- stage 3+4: attention + output projection --------
        # Units of (batch, 512-token q group), software-pipelined ACROSS
        # units:  [pairs(X)] [proj(X-1)] [tail(X)] [pairs(X+1)] [proj(X)] ...
        # pairs(X) = scores (PE) -> exp (ACT) -> attn@V (PE) at pair
        # granularity for both heads; proj(X-1) gives PE ~7us of
        # exp-independent matmuls so ACT drains stragglers; tail(X) =
        # denominator ones-matmuls (fed by vector pair/quad pre-sums of the
        # exp tiles) + reciprocal + normalize.  Diagonal tiles are
        # column-trimmed everywhere.
        with ExitStack() as s3:
            s_ps = s3.enter_context(tc.tile_pool(name="sps", bufs=2, space="PSUM"))
            y_ps = s3.enter_context(tc.tile_pool(name="yps", bufs=1, space="PSUM"))
            o_ps = s3.enter_context(tc.tile_pool(name="ops", bufs=2, space="PSUM"))
            ptp = s3.enter_context(tc.tile_pool(name="pt", bufs=12))
            psm = s3.enter_context(tc.tile_pool(name="psm", bufs=10))
            denp = s3.enter_context(tc.tile_pool(name="den", bufs=2))
            ostgp = s3.enter_context(tc.tile_pool(name="ostg", bufs=3))

            def emit_pairs(b, g, qh, yp):
                """scores/exp/attn@V pair pipeline; returns tail state."""
                q_t, k_t = qn[qh], qn[2]
                qsl = slice(b * T + g * TG, b * T + (g + 1) * TG)
                jmax = 4 * g + 3

                def issue_y(j, ap, off):
                    nc.tensor.matmul(
                        yp[:, off:] if off else yp[:],
                        vT_sb[:, b * NJB + j],
                        ap[:, off:] if off else ap[:],
                        start=(j == 0), stop=(j == jmax),
                    )

                pend, pairs, quads, diags = [], [], [], []
                for pr in range((jmax + 1) // 2):
                    sp2 = s_ps.tile([P, 2, TG], f32)
                    for jj in (0, 1):
                        j = 2 * pr + jj
                        off = (j - 4 * g) * P if j >= 4 * g else 0
                        nc.tensor.matmul(
                            sp2[:, jj, off:],
                            k_t[:, b * T + j * P: b * T + (j + 1) * P],
                            q_t[:, qsl][:, off:],
                            start=True, stop=True,
                        )
                    pt2 = ptp.tile([P, 2, TG], bf16)
                    cur = []
                    if 2 * pr >= 4 * g:
                        # diagonal pair: per-j exp on the valid range only
                        for jj in (0, 1):
                            j = 2 * pr + jj
                            off = (j - 4 * g) * P
                            nc.scalar.activation(
                                pt2[:, jj, off:], sp2[:, jj, off:],
                                AF.Exp, scale=SCALE,
                            )
                            nc.gpsimd.tensor_mul(
                                pt2[:, jj, off:off + P],
                                pt2[:, jj, off:off + P],
                                mask_sb[:, 0, 0:P],
                            )
                            cur.append((j, pt2[:, jj], off))
                            diags.append((j, pt2[:, jj], off))
                    else:
                        nc.scalar.activation(pt2[:], sp2[:], AF.Exp,
                                             scale=SCALE)
                        cur.append((2 * pr, pt2[:, 0], 0))
                        cur.append((2 * pr + 1, pt2[:, 1], 0))
                        # den partials: vector pre-sums (pair, then quad)
                        psum = psm.tile([P, TG], bf16, tag="pair")
                        nc.vector.tensor_add(psum[:], pt2[:, 0], pt2[:, 1])
                        pairs.append(psum)
                        if len(pairs) == 2:
                            qd = psm.tile([P, TG], bf16, tag="quad")
                            nc.vector.tensor_add(
                                qd[:], pairs[0][:], pairs[1][:])
                            quads.append(qd)
                            pairs = []
                    # attn@V for the previous pair while ACT runs this exp
                    if pend:
                        for (j, ap, off) in pend.pop(0):
                            issue_y(j, ap, off)
                    pend.append(cur)
                return (b, g, qh, yp, pend, quads, diags, jmax, qsl, issue_y)

            def emit_tail(st):
                """trailing attn@V + denominator + normalize for one head."""
                b, g, qh, yp, pend, quads, diags, jmax, qsl, issue_y = st
                for grp in pend:
                    for (j, ap, off) in grp:
                        issue_y(j, ap, off)
                dp = o_ps.tile([P, TG], f32, tag="op")
                first = True
                for qd in quads:
                    nc.tensor.matmul(dp[:], onesm_sb[:], qd[:],
                                     start=first, stop=False)
                    first = False
                for (j, ap, off) in diags:
                    nc.tensor.matmul(
                        dp[:, off:] if off else dp[:],
                        onesm_sb[:],
                        ap[:, off:] if off else ap[:],
                        start=first, stop=(j == jmax),
                    )
                    first = False
                den = denp.tile([P, TG], f32)
                nc.vector.reciprocal_approx_fast(den[:], dp[:])
                nc.vector.tensor_mul(yT[qh][:, qsl], yp[:], den[:])

            def emit_proj(b, g, last=False):
                for tt in range(b * (T // P) + g * 4,
                                b * (T // P) + g * 4 + 4):
                    ost = ostgp.tile([P, C], bf16)
                    for og in range(C // TG):
                        op = o_ps.tile([P, TG], f32, tag="op")
                        nc.tensor.matmul(
                            op[:], yT[0][:, tt * P:(tt + 1) * P],
                            wp_sb[:, 0, og * TG:(og + 1) * TG],
                            start=True, stop=False,
                        )
                        nc.tensor.matmul(
                            op[:], yT[1][:, tt * P:(tt + 1) * P],
                            wp_sb[:, 1, og * TG:(og + 1) * TG],
                            start=False, stop=True,
                        )
                        # copies mostly on vector (scalar stays clear for
                        # exp); final unit splits evenly to shorten the tail
                        if og % 2 if last else og == 1:
                            nc.scalar.copy(
                                ost[:, og * TG:(og + 1) * TG], op[:])
                        else:
                            nc.vector.tensor_copy(
                                ost[:, og * TG:(og + 1) * TG], op[:])
                        if last:
                            # per-og DMA so the final writes overlap copies
                            nc.sync.dma_start(
                                out=out_ap[tt * P:(tt + 1) * P,
                                           og * TG:(og + 1) * TG],
                                in_=ost[:, og * TG:(og + 1) * TG])
                    if not last:
                        nc.sync.dma_start(
                            out=out_ap[tt * P:(tt + 1) * P, :], in_=ost[:])

            units = [(b, g) for b in range(B) for g in (3, 2, 1, 0)]
            prev = None
            for (b, g) in units:
                yp0 = y_ps.tile([P, TG], f32, tag="yp0")
                yp1 = y_ps.tile([P, TG], f32, tag="yp1")
                st0 = emit_pairs(b, g, 0, yp0)
                st1 = emit_pairs(b, g, 1, yp1)
                # head-0 tail first so its recip/normalize enter the vector
                # queue ahead of the previous unit's 12 projection copies
                emit_tail(st0)
                if prev is not None:
                    emit_proj(*prev)
                emit_tail(st1)
                prev = (b, g)
            emit_proj(*prev, last=True)

def build_nc():
    """Build and compile the (single, shared across cores) Bass program."""
    if "nc" in _CACHE:
        return _CACHE["nc"]
    import concourse.mybir as mybir
    import concourse.tile as tile
    from concourse import bacc

    f32 = mybir.dt.float32  # noqa: F841
    bf16 = mybir.dt.bfloat16

    nc = bacc.Bacc("TRN2", target_bir_lowering=False, debug=False)
    shapes = {
        "x_sw": ((P, NT, KT, TG), bf16),
        "wq_sw": ((P, 4, KT, P), bf16),
        "wp_sw": ((P, QH_PER_CORE, C), bf16),
        "cs_sw": ((P, 2, T), bf16),
        "mask_sw": ((P, 4, TG), bf16),
        "eye_sw": ((P, P), bf16),
        "ones_sw": ((P, 1), bf16),
    }
    t_in = {
        name: nc.dram_tensor(name, shape, dt, kind="ExternalInput").ap()
        for name, (shape, dt) in shapes.items()
    }
    out_ap = nc.dram_tensor("out", (TOK, C), bf16, kind="ExternalOutput").ap()

    with tile.TileContext(nc) as tc:
        _emit(tc, out_ap, t_in)
    nc.compile()
    _CACHE["nc"] = nc
    return nc


# --------------------------------------------------------------------------
# host-side data preparation
# --------------------------------------------------------------------------

def _swizzle_ktiles(a2d):
    """[R*128, F] -> [128, R, F] picking partition-within-tile as leading."""
    r128, f = a2d.shape
    r = r128 // P
    return np.ascontiguousarray(a2d.reshape(r, P, f).transpose(1, 0, 2))


def host_prep(x, w_attn, w_proj, cos, sin):
    x = np.asarray(x, np.float32)
    w_attn = np.asarray(w_attn, np.float32)
    w_proj = np.asarray(w_proj, np.float32)
    cos = np.asarray(cos, np.float32).reshape(T, HD // 2)
    sin = np.asarray(sin, np.float32).reshape(T, HD // 2)

    # x: (B,T,C) -> xT (C, TOK) -> [128, n, k, t]
    xT = x.reshape(TOK, C).T                        # (C, TOK)
    x_sw = (
        xT.reshape(KT, P, NT, TG).transpose(1, 2, 0, 3)  # (P, n, k, t)
    )
    x_sw = np.ascontiguousarray(x_sw).astype(BF16)

    # cos/sin duplicated across both 64-partition halves: [128, 2, T]
    c2 = np.concatenate([cos.T, cos.T], axis=0)     # (128, T)
    s2 = np.concatenate([sin.T, -sin.T], axis=0)    # sign-folded for rope add
    cs_sw = np.stack([c2, s2], axis=1).astype(BF16)  # (128, 2, T)

    # causal masks for the 4 diagonal offsets: keep col >= row + 128*off
    col = np.arange(TG)[None, :]
    row = np.arange(P)[:, None]
    mask_sw = np.stack(
        [(col >= row + P * off) for off in range(4)], axis=1
    ).astype(BF16)                                   # (128, 4, 512)

    eye_sw = np.eye(P, dtype=np.float32).astype(BF16)
    ones_sw = np.ones((P, 1), np.float32).astype(BF16)

    in_maps = []
    for c in range(N_CORES):
        qrows = w_attn[QH_PER_CORE * HD * c: QH_PER_CORE * HD * (c + 1)]
        krows = w_attn[C + HD * c: C + HD * (c + 1)]
        vrows = w_attn[C + KV_DIM + HD * c: C + KV_DIM + HD * (c + 1)]
        w_sel = np.concatenate([qrows, krows, vrows], axis=0)   # (512, C)
        wq_sw = _swizzle_ktiles(w_sel.T).astype(BF16)           # (128, 16, 512)
        # [p, k, m*128+c] -> [p, m, k, c] (per-m contiguous for startup DMA)
        wq_sw = np.ascontiguousarray(
            wq_sw.reshape(P, KT, 4, P).transpose(0, 2, 1, 3))

        wp_sel = w_proj[:, QH_PER_CORE * HD * c: QH_PER_CORE * HD * (c + 1)]
        wp_sw = _swizzle_ktiles(np.ascontiguousarray(wp_sel.T)).astype(BF16)

        in_maps.append({
            "x_sw": x_sw,
            "wq_sw": wq_sw,
            "wp_sw": np.ascontiguousarray(wp_sw.reshape(P, QH_PER_CORE, C)),
            "cs_sw": cs_sw,
            "mask_sw": mask_sw,
            "eye_sw": eye_sw,
            "ones_sw": ones_sw,
        })
    return in_maps


def run_on_hw(in_maps, trace=False, **kwargs):
    from concourse import bass_utils

    nc = build_nc()
    return bass_utils.run_bass_kernel_spmd(
        nc, in_maps, core_ids=list(range(N_CORES)), trace=trace, **kwargs
    )


def kernel(x, w_attn, w_proj, cos, sin):
    in_maps = host_prep(x, w_attn, w_proj, cos, sin)
    res = run_on_hw(in_maps)
    out = np.zeros((TOK, C), np.float64)
    for r in res.results:
        out += r["out"].astype(np.float64)
    return out.astype(np.float32).reshape(B, T, C)



# revision 46
# speedup vs baseline: 1.1740x; 1.0038x over previous
"""Trainium2 Bass kernel for nn_CausalSelfAttention (GQA + RoPE + qk-RMSNorm).

Strategy (Megatron-style head parallelism over 8 NeuronCores):
  - Each core owns 2 of the 16 q heads and the matching 1 of 8 kv heads.
  - Per core: QKV projection for its 512 rows of w_attn, RoPE + qk RMS norm,
    causal flash-style attention for its (2 q heads x 2 batches), and a
    partial output projection through its 256 columns of w_proj.
  - Host sums the 8 partial outputs (no on-device collectives).

All tensors are fed to the device pre-swizzled into SBUF-ready
[128, free...] layouts (bf16 for matmul operands).  Matmuls run in bf16 with
fp32 PSUM accumulation; softmax/statistics run in fp32.

Self-contained: hardcodes all shapes from the problem spec.
"""

import math
import numpy as np
import ml_dtypes
from contextlib import ExitStack

# ---- problem constants (hardcoded per spec) ----
B, T, C = 2, 2048, 2048
N_HEAD, N_KV_HEAD, HD = 16, 8, 128
KV_DIM = N_KV_HEAD * HD
EPS = 1.1920929e-07
N_CORES = 8
QH_PER_CORE = N_HEAD // N_CORES          # 2
TOK = B * T                              # 4096
P = 128
TG = 512                                 # token group (matmul N)
NT = TOK // TG                           # 8 token groups
KT = C // P                              # 16 contraction tiles
NGB = T // TG                            # 4 q groups per batch
NJB = T // P                             # 16 k tiles per batch
SCALE = 1.0 / math.sqrt(HD)

BF16 = ml_dtypes.bfloat16

_CACHE = {}


# --------------------------------------------------------------------------
# device program
# --------------------------------------------------------------------------

def _emit(tc, out_ap, t_in):
    import concourse.bass as bass  # noqa: F401
    import concourse.mybir as mybir

    f32 = mybir.dt.float32
    bf16 = mybir.dt.bfloat16
    AF = mybir.ActivationFunctionType
    nc = tc.nc

    x_d = t_in["x_sw"]
    wq_d = t_in["wq_sw"]
    wp_d = t_in["wp_sw"]
    cs_d = t_in["cs_sw"]
    mask_d = t_in["mask_sw"]
    eye_d = t_in["eye_sw"]
    ones_d = t_in["ones_sw"]

    with ExitStack() as root:
        const = root.enter_context(tc.tile_pool(name="const", bufs=1))
        # startup: interleave x(n=0) k-chunks with per-m weight chunks so the
        # first QKV chain starts ~2.5us in and is never DMA-gated afterwards.
        # wq layout is [P, m, k, 128]: per-m slices are contiguous.
        wq_sb = const.tile([P, 4, KT, P], bf16)
        x0_sb = const.tile([P, KT, TG], bf16, tag="x0")
        # finest-grain interleave for the first two chains (m=2 and m=0 run
        # in half-k alternation, so PE has work while x0's tail streams in)
        nc.sync.dma_start(out=x0_sb[:, 0:4, :], in_=x_d[:, 0, 0:4, :])
        nc.sync.dma_start(out=wq_sb[:, 2, 0:8], in_=wq_d[:, 2, 0:8])
        nc.sync.dma_start(out=wq_sb[:, 0, 0:8], in_=wq_d[:, 0, 0:8])
        nc.sync.dma_start(out=x0_sb[:, 4:8, :], in_=x_d[:, 0, 4:8, :])
        nc.sync.dma_start(out=wq_sb[:, 2, 8:16], in_=wq_d[:, 2, 8:16])
        nc.sync.dma_start(out=wq_sb[:, 0, 8:16], in_=wq_d[:, 0, 8:16])
        nc.sync.dma_start(out=x0_sb[:, 8:16, :], in_=x_d[:, 0, 8:16, :])
        nc.sync.dma_start(out=wq_sb[:, 1], in_=wq_d[:, 1])
        nc.sync.dma_start(out=wq_sb[:, 3], in_=wq_d[:, 3])
        ones_sb = const.tile([P, 1], bf16)
        nc.sync.dma_start(out=ones_sb[:], in_=ones_d)
        eye_sb = const.tile([P, P], bf16)
        nc.sync.dma_start(out=eye_sb[:], in_=eye_d)
        cs_sb = const.tile([P, 2, T], bf16)
        nc.sync.dma_start(out=cs_sb[:], in_=cs_d)
        mask_sb = const.tile([P, 4, TG], bf16)
        nc.sync.dma_start(out=mask_sb[:], in_=mask_d)
        wp_sb = const.tile([P, QH_PER_CORE, C], bf16)
        nc.sync.dma_start(out=wp_sb[:], in_=wp_d)
        eps_sb = const.tile([P, 1], f32)
        nc.vector.memset(eps_sb[:], EPS)
        onesm_sb = const.tile([P, P], bf16)
        nc.vector.memset(onesm_sb[:], 1.0)

        big = root.enter_context(tc.tile_pool(name="big", bufs=1))
        # post-rope, post-norm q (2 heads) and k, in [d, tok] layout
        qn = [big.tile([P, TOK], bf16, name=f"qn{m}", tag=f"qn{m}") for m in range(3)]
        v_sb = big.tile([P, TOK], bf16, tag="v")
        vT_sb = big.tile([P, 2 * NJB, P], bf16, tag="vT")   # [ktok, (b,j), d]
        yT = [big.tile([P, TOK], bf16, name=f"yT{h}", tag=f"yT{h}") for h in range(QH_PER_CORE)]

        # ------- stage 1+2: QKV projection + rope/norm + v transpose -------
        # rope runs at (n, m) granularity right after each normalized tile so
        # nothing serializes at the batch boundary before attention starts.
        with ExitStack() as s1:
            xin = s1.enter_context(tc.tile_pool(name="xin", bufs=2))
            qkv_ps = s1.enter_context(tc.tile_pool(name="qkvps", bufs=3, space="PSUM"))
            vt_ps = s1.enter_context(tc.tile_pool(name="vtps", bufs=3, space="PSUM"))
            ssq_ps = s1.enter_context(tc.tile_pool(name="ssqps", bufs=2, space="PSUM"))
            sqp = s1.enter_context(tc.tile_pool(name="sq", bufs=3))
            srp = s1.enter_context(tc.tile_pool(name="sr", bufs=3))
            ropet = s1.enter_context(tc.tile_pool(name="ropet", bufs=4))

            for b in range(B):
                for nn in range(NT // B):
                    n = b * (NT // B) + nn
                    tsl = slice(n * TG, (n + 1) * TG)       # token cols in qn
                    csl = slice(nn * TG, (nn + 1) * TG)     # cos/sin cols
                    if n == 0:
                        xb = x0_sb
                    else:
                        xb = xin.tile([P, KT, TG], bf16)
                        nc.sync.dma_start(out=xb[:, 0:8, :], in_=x_d[:, n, 0:8, :])
                        nc.sync.dma_start(out=xb[:, 8:16, :], in_=x_d[:, n, 8:16, :])
                    rope_pend = []

                    def _rope_flush(lo=2):
                        # finish rope (xsw*sin + add) one m-step late, so the
                        # vector queue never head-blocks on the half-swap DMA
                        while len(rope_pend) >= lo:
                            mm_, t1_, xsw_, tsl_ = rope_pend.pop(0)
                            nc.vector.tensor_mul(
                                xsw_[:], xsw_[:], cs_sb[:, 1, csl])
                            nc.vector.tensor_add(
                                qn[mm_][:, tsl_], t1_[:], xsw_[:])

                    def _chain(m, ps, ks):
                        for k in ks:
                            nc.tensor.matmul(
                                ps[:],
                                wq_sb[:, m, k],
                                xb[:, k],
                                start=(k == 0),
                                stop=(k == KT - 1),
                            )

                    n0_ps = {}
                    if n == 0:
                        # half-k interleave of the first two chains: PE stays
                        # fed while the second half of x0 is still in flight
                        for m in (2, 0):
                            n0_ps[m] = qkv_ps.tile([P, TG], f32, name="ps")
                            _chain(m, n0_ps[m], range(0, 8))
                        for m in (2, 0):
                            _chain(m, n0_ps[m], range(8, KT))

                    for m in (2, 0, 1, 3):
                        if m in n0_ps:
                            ps = n0_ps[m]
                        else:
                            ps = qkv_ps.tile([P, TG], f32, name="ps")
                            _chain(m, ps, range(KT))
                        if m == 3:
                            # off the (busy) vector queue; ACT reads PSUM fast
                            nc.scalar.copy(v_sb[:, tsl], ps[:])
                        else:
                            # rms-norm: broadcast sum-of-squares via all-ones MM
                            sq = sqp.tile([P, TG], bf16)
                            nc.scalar.activation(sq[:], ps[:], AF.Square)
                            ssqb = ssq_ps.tile([P, TG], f32)
                            nc.tensor.matmul(
                                ssqb[:], onesm_sb[:], sq[:], start=True, stop=True
                            )
                            srb = srp.tile([P, TG], f32)
                            nc.scalar.activation(
                                srb[:], ssqb[:], AF.Sqrt,
                                bias=eps_sb[:], scale=1.0 / HD,
                            )
                            nc.vector.reciprocal_approx_fast(srb[:], srb[:])
                            # normalized copy psum -> sbuf (rope rotation
                            # commutes with the per-token scale)
                            nc.vector.tensor_mul(qn[m][:, tsl], ps[:], srb[:])
                            # rope on this 512-token tile; the half-swap runs
                            # on the sync HWDGE queue, t1 = [x1*c ; x2*c] now,
                            # xsw*s2n = [x2*s ; -x1*s] deferred one m-step
                            t1 = ropet.tile([P, TG], bf16, tag="t1")
                            xsw = ropet.tile([P, TG], bf16, tag="xsw")
                            nc.gpsimd.dma_start(
                                out=xsw[0:64, :], in_=qn[m][64:128, tsl])
                            nc.gpsimd.dma_start(
                                out=xsw[64:128, :], in_=qn[m][0:64, tsl])
                            nc.vector.tensor_mul(
                                t1[:], qn[m][:, tsl], cs_sb[:, 0, csl])
                            rope_pend.append((m, t1, xsw, tsl))
                            _rope_flush(2)
                        if m == 3:
                            _rope_flush(1)
                    # v transposes deferred by one n group so PE never waits
                    # on the v copy trailing in the scalar queue
                    def _vt(nv):
                        for blk in range(nv * (TG // P), (nv + 1) * (TG // P)):
                            tp = vt_ps.tile([P, P], bf16)
                            nc.tensor.transpose(
                                tp[:], v_sb[:, blk * P:(blk + 1) * P], eye_sb[:]
                            )
                            if blk % 2:
                                nc.vector.tensor_copy(vT_sb[:, blk], tp[:])
                            else:
                                nc.scalar.copy(vT_sb[:, blk], tp[:])
                    if n > 0:
                        _vt(n - 1)
                    if n == NT - 1:
                        _vt(n)

        # ---------------- stage 3+4: attention + output projection --------
        # Units of (batch, 512-token q group), software-pipelined ACROSS
        # units:  [pairs(X)] [proj(X-1)] [tail(X)] [pairs(X+1)] [proj(X)] ...
        # pairs(X) = scores (PE) -> exp (ACT) -> attn@V (PE) at pair
        # granularity for both heads; proj(X-1) gives PE ~7us of
        # exp-independent matmuls so ACT drains stragglers; tail(X) =
        # denominator ones-matmuls (fed by vector pair/quad pre-sums of the
        # exp tiles) + reciprocal + normalize.  Diagonal tiles are
        # column-trimmed everywhere.
        with ExitStack() as s3:
            s_ps = s3.enter_context(tc.tile_pool(name="sps", bufs=2, space="PSUM"))
            y_ps = s3.enter_context(tc.tile_pool(name="yps", bufs=1, space="PSUM"))
            o_ps = s3.enter_context(tc.tile_pool(name="ops", bufs=2, space="PSUM"))
            ptp = s3.enter_context(tc.tile_pool(name="pt", bufs=12))
            psm = s3.enter_context(tc.tile_pool(name="psm", bufs=10))
            denp = s3.enter_context(tc.tile_pool(name="den", bufs=2))
            ostgp = s3.enter_context(tc.tile_pool(name="ostg", bufs=3))

            def emit_pairs(b, g, qh, yp):
                """scores/exp/attn@V pair pipeline; returns tail state."""
                q_t, k_t = qn[qh], qn[2]
                qsl = slice(b * T + g * TG, b * T + (g + 1) * TG)
                jmax = 4 * g + 3

                def issue_y(j, ap, off):
                    nc.tensor.matmul(
                        yp[:, off:] if off else yp[:],
                        vT_sb[:, b * NJB + j],
                        ap[:, off:] if off else ap[:],
                        start=(j == 0), stop=(j == jmax),
                    )

                pend, pairs, quads, diags = [], [], [], []
                for pr in range((jmax + 1) // 2):
                    sp2 = s_ps.tile([P, 2, TG], f32)
                    for jj in (0, 1):
                        j = 2 * pr + jj
                        off = (j - 4 * g) * P if j >= 4 * g else 0
                        nc.tensor.matmul(
                            sp2[:, jj, off:],
                            k_t[:, b * T + j * P: b * T + (j + 1) * P],
                            q_t[:, qsl][:, off:],
                            start=True, stop=True,
                        )
                    pt2 = ptp.tile([P, 2, TG], bf16)
                    cur = []
                    if 2 * pr >= 4 * g:
                        # diagonal pair: per-j exp on the valid range only
                        for jj in (0, 1):
                            j = 2 * pr + jj
                            off = (j - 4 * g) * P
                            nc.scalar.activation(
                                pt2[:, jj, off:], sp2[:, jj, off:],
                                AF.Exp, scale=SCALE,
                            )
                            nc.gpsimd.tensor_mul(
                                pt2[:, jj, off:off + P],
                                pt2[:, jj, off:off + P],
                                mask_sb[:, 0, 0:P],
                            )
                            cur.append((j, pt2[:, jj], off))
                            diags.append((j, pt2[:, jj], off))
                    else:
                        nc.scalar.activation(pt2[:], sp2[:], AF.Exp,
                                             scale=SCALE)
                        cur.append((2 * pr, pt2[:, 0], 0))
                        cur.append((2 * pr + 1, pt2[:, 1], 0))
                        # den partials: vector pre-sums (pair, then quad)
                        psum = psm.tile([P, TG], bf16, tag="pair")
                        nc.vector.tensor_add(psum[:], pt2[:, 0], pt2[:, 1])
                        pairs.append(psum)
                        if len(pairs) == 2:
                            qd = psm.tile([P, TG], bf16, tag="quad")
                            nc.vector.tensor_add(
                                qd[:], pairs[0][:], pairs[1][:])
                            quads.append(qd)
                            pairs = []
                    # attn@V for the previous pair while ACT runs this exp
                    if pend:
                        for (j, ap, off) in pend.pop(0):
                            issue_y(j, ap, off)
                    pend.append(cur)
                return (b, g, qh, yp, pend, quads, diags, jmax, qsl, issue_y)

            def emit_tail(st):
                """trailing attn@V + denominator + normalize for one head."""
                b, g, qh, yp, pend, quads, diags, jmax, qsl, issue_y = st
                for grp in pend:
                    for (j, ap, off) in grp:
                        issue_y(j, ap, off)
                dp = o_ps.tile([P, TG], f32, tag="op")
                first = True
                for qd in quads:
                    nc.tensor.matmul(dp[:], onesm_sb[:], qd[:],
                                     start=first, stop=False)
                    first = False
                for (j, ap, off) in diags:
                    nc.tensor.matmul(
                        dp[:, off:] if off else dp[:],
                        onesm_sb[:],
                        ap[:, off:] if off else ap[:],
                        start=first, stop=(j == jmax),
                    )
                    first = False
                den = denp.tile([P, TG], f32)
                nc.vector.reciprocal_approx_fast(den[:], dp[:])
                nc.vector.tensor_mul(yT[qh][:, qsl], yp[:], den[:])

            def emit_proj(b, g, last=False):
                dense = (b == B - 1 and g <= 1) or last
                for tt in range(b * (T // P) + g * 4,
                                b * (T // P) + g * 4 + 4):
                    ost = ostgp.tile([P, C], bf16)
                    for og in range(C // TG):
                        op = o_ps.tile([P, TG], f32, tag="op")
                        nc.tensor.matmul(
                            op[:], yT[0][:, tt * P:(tt + 1) * P],
                            wp_sb[:, 0, og * TG:(og + 1) * TG],
                            start=True, stop=False,
                        )
                        nc.tensor.matmul(
                            op[:], yT[1][:, tt * P:(tt + 1) * P],
                            wp_sb[:, 1, og * TG:(og + 1) * TG],
                            start=False, stop=True,
                        )
                        # copies mostly on vector (scalar stays clear for
                        # exp); final unit splits evenly to shorten the tail
                        if og % 2 if dense else og == 1:
                            nc.scalar.copy(
                                ost[:, og * TG:(og + 1) * TG], op[:])
                        else:
                            nc.vector.tensor_copy(
                                ost[:, og * TG:(og + 1) * TG], op[:])
                        if last:
                            # per-og DMA so the final writes overlap copies
                            nc.sync.dma_start(
                                out=out_ap[tt * P:(tt + 1) * P,
                                           og * TG:(og + 1) * TG],
                                in_=ost[:, og * TG:(og + 1) * TG])
                    if not last:
                        nc.sync.dma_start(
                            out=out_ap[tt * P:(tt + 1) * P, :], in_=ost[:])

            units = [(b, g) for b in range(B) for g in (3, 2, 1, 0)]
            prev = None
            for (b, g) in units:
                yp0 = y_ps.tile([P, TG], f32, tag="yp0")
                yp1 = y_ps.tile([P, TG], f32, tag="yp1")
                st0 = emit_pairs(b, g, 0, yp0)
                st1 = emit_pairs(b, g, 1, yp1)
                # head-0 tail first so its recip/normalize enter the vector
                # queue ahead of the previous unit's 12 projection copies
                emit_tail(st0)
                if prev is not None:
                    emit_proj(*prev)
                emit_tail(st1)
                prev = (b, g)
            emit_proj(*prev, last=True)

def build_nc():
    """Build and compile the (single, shared across cores) Bass program."""
    if "nc" in _CACHE:
        return _CACHE["nc"]
    import concourse.mybir as mybir
    import concourse.tile as tile
    from concourse import bacc

    f32 = mybir.dt.float32  # noqa: F841
    bf16 = mybir.dt.bfloat16

    nc = bacc.Bacc("TRN2", target_bir_lowering=False, debug=False)
    shapes = {
        "x_sw": ((P, NT, KT, TG), bf16),
        "wq_sw": ((P, 4, KT, P), bf16),
        "wp_sw": ((P, QH_PER_CORE, C), bf16),
        "cs_sw": ((P, 2, T), bf16),
        "mask_sw": ((P, 4, TG), bf16),
        "eye_sw": ((P, P), bf16),
        "ones_sw": ((P, 1), bf16),
    }
    t_in = {
        name: nc.dram_tensor(name, shape, dt, kind="ExternalInput").ap()
        for name, (shape, dt) in shapes.items()
    }
    out_ap = nc.dram_tensor("out", (TOK, C), bf16, kind="ExternalOutput").ap()

    with tile.TileContext(nc) as tc:
        _emit(tc, out_ap, t_in)
    nc.compile()
    _CACHE["nc"] = nc
    return nc


# --------------------------------------------------------------------------
# host-side data preparation
# --------------------------------------------------------------------------

def _swizzle_ktiles(a2d):
    """[R*128, F] -> [128, R, F] picking partition-within-tile as leading."""
    r128, f = a2d.shape
    r = r128 // P
    return np.ascontiguousarray(a2d.reshape(r, P, f).transpose(1, 0, 2))


def host_prep(x, w_attn, w_proj, cos, sin):
    x = np.asarray(x, np.float32)
    w_attn = np.asarray(w_attn, np.float32)
    w_proj = np.asarray(w_proj, np.float32)
    cos = np.asarray(cos, np.float32).reshape(T, HD // 2)
    sin = np.asarray(sin, np.float32).reshape(T, HD // 2)

    # x: (B,T,C) -> xT (C, TOK) -> [128, n, k, t]
    xT = x.reshape(TOK, C).T                        # (C, TOK)
    x_sw = (
        xT.reshape(KT, P, NT, TG).transpose(1, 2, 0, 3)  # (P, n, k, t)
    )
    x_sw = np.ascontiguousarray(x_sw).astype(BF16)

    # cos/sin duplicated across both 64-partition halves: [128, 2, T]
    c2 = np.concatenate([cos.T, cos.T], axis=0)     # (128, T)
    s2 = np.concatenate([sin.T, -sin.T], axis=0)    # sign-folded for rope add
    cs_sw = np.stack([c2, s2], axis=1).astype(BF16)  # (128, 2, T)

    # causal masks for the 4 diagonal offsets: keep col >= row + 128*off
    col = np.arange(TG)[None, :]
    row = np.arange(P)[:, None]
    mask_sw = np.stack(
        [(col >= row + P * off) for off in range(4)], axis=1
    ).astype(BF16)                                   # (128, 4, 512)

    eye_sw = np.eye(P, dtype=np.float32).astype(BF16)
    ones_sw = np.ones((P, 1), np.float32).astype(BF16)

    in_maps = []
    for c in range(N_CORES):
        qrows = w_attn[QH_PER_CORE * HD * c: QH_PER_CORE * HD * (c + 1)]
        krows = w_attn[C + HD * c: C + HD * (c + 1)]
        vrows = w_attn[C + KV_DIM + HD * c: C + KV_DIM + HD * (c + 1)]
        w_sel = np.concatenate([qrows, krows, vrows], axis=0)   # (512, C)
        wq_sw = _swizzle_ktiles(w_sel.T).astype(BF16)           # (128, 16, 512)
        # [p, k, m*128+c] -> [p, m, k, c] (per-m contiguous for startup DMA)
        wq_sw = np.ascontiguousarray(
            wq_sw.reshape(P, KT, 4, P).transpose(0, 2, 1, 3))

        wp_sel = w_proj[:, QH_PER_CORE * HD * c: QH_PER_CORE * HD * (c + 1)]
        wp_sw = _swizzle_ktiles(np.ascontiguousarray(wp_sel.T)).astype(BF16)

        in_maps.append({
            "x_sw": x_sw,
            "wq_sw": wq_sw,
            "wp_sw": np.ascontiguousarray(wp_sw.reshape(P, QH_PER_CORE, C)),
            "cs_sw": cs_sw,
            "mask_sw": mask_sw,
            "eye_sw": eye_sw,
            "ones_sw": ones_sw,
        })
    return in_maps


def run_on_hw(in_maps, trace=False, **kwargs):
    from concourse import bass_utils

    nc = build_nc()
    return bass_utils.run_bass_kernel_spmd(
        nc, in_maps, core_ids=list(range(N_CORES)), trace=trace, **kwargs
    )


def kernel(x, w_attn, w_proj, cos, sin):
    in_maps = host_prep(x, w_attn, w_proj, cos, sin)
    res = run_on_hw(in_maps)
    out = np.zeros((TOK, C), np.float64)
    for r in res.results:
        out += r["out"].astype(np.float64)
    return out.astype(np.float32).reshape(B, T, C)

